# revision 62
# baseline (speedup 1.0000x reference)
"""Trainium2 Bass kernel for the analog-crossbar CustomLayer (v2).

Math (per 512x512 weight tile, per reference.py):
    cond   = (w - wmin)*s + G_MIN ; quantize to 16 levels (lev in 0..15)
    g_eff  = 1/(1/cond + r_wire)          (Jeong nonlinear IV model)
    cur    = x @ g_eff ; ideal = x @ cond = step*(x @ lev) + G_MIN*rowsum(x)
    out    = ((cur - mean(cur))*coeff + mean(ideal) - offset)/s summed over
             in_tiles, plus bias; coeff from per-row ranges of ideal/cur.

v2 strategy:
  - The weight-static transform (mapping -> quantize -> Jeong) is precomputed
    on the host in float32 matching the reference op order. The device gets
    g16 = fp16(2^13 * g_eff) (scaled into fp16 normal range) and
    rl16 = fp16(lev) (0..15, exact in fp16).
  - Both tile matmuls run in fp16 at full PE speed; PSUM accumulates fp32.
  - Per-tile row stats: Act drains PSUM -> SBUF (f32) with free accum row
    sums; Pool pre-folds 512->256 for max/min; DVE finishes the reduces over
    a combined [cur|ideal] buffer.
  - The correction scale A is applied inside the PE in_tile accumulation via
    a diagonal-lhs matmul (diag(A) built by one Act op per tile from the
    identity); the per-row additive D collapses across tiles and is applied
    with the bias in one final scalar_tensor_tensor.
  - Epilogue of chunk m is emitted after chunk m+1's matmuls (software
    pipelining) so the PE stream never waits on the stats chain.

Sharding: out_tiles (columns of weight) across 8 cores; x replicated.
Each core computes a [1024, 512] slice; host concatenates.
"""

import numpy as np
import sys

sys.path.insert(0, "/opt/trn_rl_repo")

# ---- problem constants (hardcoded; must match reference) ----
R_HRS = 1.0e6
R_LRS = 1.0e4
RP = 2.0
BITS = 4
TS = 512
G_MIN = np.float32(1.0 / R_HRS)
G_MAX = np.float32(1.0 / R_LRS)
B = 1024          # batch
IN_F = 4096       # in features
OUT_F = 4096      # out features
NCORES = 8
IT = IN_F // TS   # 8 in tiles
KC = TS // 128    # 4 k-chunks per tile
MB = B // 128     # 8 batch chunks
CS = 8192.0       # 2**13: fp16 scaling of conductances

_CACHE = {}


def _build():
    import concourse.bass as bass
    import concourse.tile as tile
    from concourse import bacc, mybir

    f32 = mybir.dt.float32
    f32r = mybir.dt.float32r
    f16 = mybir.dt.float16
    f8 = mybir.dt.float8e4
    Alu = mybir.AluOpType
    Act = mybir.ActivationFunctionType

    nc = bacc.Bacc(None, target_bir_lowering=False, debug=False)

    xt_d = nc.dram_tensor("xq16", [MB, 128, IT * KC * 128], f16,
                          kind="ExternalInput")
    g_d = nc.dram_tensor("g16", [128, IT * KC * TS], f16, kind="ExternalInput")
    rl_d = nc.dram_tensor("rl8", [128, IT * KC * TS], f8, kind="ExternalInput")
    x8_d = nc.dram_tensor("xq8", [MB, 128, IT * KC * 128], f8,
                          kind="ExternalInput")
    rsum_d = nc.dram_tensor("rsum", [128, MB * IT], f32, kind="ExternalInput")
    kon_d = nc.dram_tensor("kon", [128, 3 * IT], f32, kind="ExternalInput")
    biasb_d = nc.dram_tensor("biasb", [128, TS], f32, kind="ExternalInput")
    id_d = nc.dram_tensor("ident", [128, 128], f32, kind="ExternalInput")
    out_d = nc.dram_tensor("out", [B, TS], f32, kind="ExternalOutput")



    with tile.TileContext(nc) as tc:
        with (
            tc.tile_pool(name="const", bufs=1) as constp,
            tc.tile_pool(name="gq", bufs=1) as gqp,
            tc.tile_pool(name="xm", bufs=4) as xmp,
            tc.tile_pool(name="combo", bufs=3) as combop,
            tc.tile_pool(name="fold", bufs=3) as foldp,
            tc.tile_pool(name="stats", bufs=3) as statp,
            tc.tile_pool(name="diag", bufs=2) as diagp,
            tc.tile_pool(name="outsb", bufs=2) as outp,
            tc.tile_pool(name="psA", bufs=3, space=bass.MemorySpace.PSUM) as psAp,
            tc.tile_pool(name="psB", bufs=3, space=bass.MemorySpace.PSUM) as psBp,
            tc.tile_pool(name="psO", bufs=2, space=bass.MemorySpace.PSUM) as psOp,
        ):
            # small constants go on the Pool SWDGE queue so they never
            # occupy the HWDGE ring ahead of the startup-critical loads
            kon_sb = constp.tile([128, 3 * IT], f32)
            nc.gpsimd.dma_start(out=kon_sb[:], in_=kon_d.ap()[:])
            biasb_sb = constp.tile([128, TS], f32)
            nc.gpsimd.dma_start(out=biasb_sb[:], in_=biasb_d.ap()[:])
            id_sb = constp.tile([128, 128], f16)
            nc.gpsimd.dma_start(out=id_sb[:], in_=id_d.ap()[:])

            # DMA front order matches the pair's cur-then-ideal phase order:
            # x chunk 0, all g tables (cur matmuls chase this stream), x
            # chunk 1, then all level tables (ideal matmuls chase those),
            # and the late-needed row sums last
            xm0 = xmp.tile([128, IT * KC, 128], f16, tag="xm")
            nc.sync.dma_start(
                out=xm0[:].rearrange("p c m -> p (c m)"), in_=xt_d.ap()[0])

            g_sb = gqp.tile([128, IT * KC * TS], f16)
            rl_sb = gqp.tile([128, IT * KC * TS], f8)
            xm1 = xmp.tile([128, IT * KC, 128], f16, tag="xm")
            xm1f = xm1[:].rearrange("p c m -> p (c m)")
            XH = IT * KC * 128 // 2
            for it in range(IT):
                sl = slice(it * KC * TS, (it + 1) * KC * TS)
                nc.sync.dma_start(out=g_sb[:, sl], in_=g_d.ap()[:, sl])
                if it == 0:
                    nc.sync.dma_start(out=xm1f[:, 0:XH],
                                      in_=xt_d.ap()[1][:, 0:XH])
                elif it == 2:
                    nc.sync.dma_start(out=xm1f[:, XH:],
                                      in_=xt_d.ap()[1][:, XH:])
            xm0_8 = xmp.tile([128, IT * KC, 128], f8, tag="xm8")
            nc.sync.dma_start(
                out=xm0_8[:].rearrange("p c m -> p (c m)"), in_=x8_d.ap()[0])
            xm1_8 = xmp.tile([128, IT * KC, 128], f8, tag="xm8")
            nc.sync.dma_start(
                out=xm1_8[:].rearrange("p c m -> p (c m)"), in_=x8_d.ap()[1])
            for it in range(IT):
                sl = slice(it * KC * TS, (it + 1) * KC * TS)
                nc.sync.dma_start(out=rl_sb[:, sl], in_=rl_d.ap()[:, sl])
            rsum_sb = constp.tile([128, MB * IT], f32)
            nc.sync.dma_start(out=rsum_sb[:], in_=rsum_d.ap()[:])

            kA = kon_sb[:, 0:IT]           # step/s
            kZ = kon_sb[:, IT:2 * IT]      # step/(512*s)
            kW = kon_sb[:, 2 * IT:3 * IT]  # wmin

            def emit_epilogue(m, combo, Ab, Dt):
                diag = diagp.tile([128, IT * 128], f16, tag="diag")
                for it in range(IT):
                    nc.vector.tensor_scalar(out=diag[:, it * 128:(it + 1) * 128],
                                            in0=id_sb[:],
                                            scalar1=Ab[:, it:it + 1],
                                            scalar2=None, op0=Alu.mult)
                out_ps = psOp.tile([128, TS], f32, tag="out_ps")
                for it in range(IT):
                    nc.tensor.matmul(out_ps[:], diag[:, it * 128:(it + 1) * 128],
                                     combo[:, it, 0:TS],
                                     start=(it == 0), stop=(it == IT - 1))
                osb = outp.tile([128, TS], f32, tag="osb")
                nc.vector.scalar_tensor_tensor(out=osb[:], in0=out_ps[:],
                                               scalar=Dt[:, 0:1], in1=biasb_sb[:],
                                               op0=Alu.add, op1=Alu.add)
                nc.sync.dma_start(out=out_d.ap()[m * 128:(m + 1) * 128, :],
                                  in_=osb[:])

            HF = IT // 2   # tiles per fold batch
            H = TS // 2
            Q = TS // 4

            class Chunk:
                """Per-batch-chunk emission state + helpers."""

                def __init__(self, m, xm, xm8, last=False):
                    self.m = m
                    self.xm = xm
                    self.xm8 = xm8
                    self.last = last
                    self.rs = rsum_sb[:, m * IT:(m + 1) * IT]
                    self.combo = combop.tile([128, IT, 2 * TS], f16, tag="combo")
                    self.zsum = statp.tile([128, IT], f32, tag="zsum")
                    if last:
                        self.csum = statp.tile([128, IT], f32, tag="csum")
                        self.A2 = statp.tile([128, IT], f32, tag="A2")
                    # interleaved: col 2t = cur stat, col 2t+1 = ideal stat
                    self.smax = statp.tile([128, 2 * IT], f32, tag="smax")
                    self.smin = statp.tile([128, 2 * IT], f32, tag="smin")
                    for nm in ("dz", "dc", "co", "Ab", "Db", "t3"):
                        setattr(self, nm,
                                statp.tile([128, IT], f32, tag=nm, name=nm))
                    self.Dt = statp.tile([128, 1], f32, tag="Dt")
                    self.u = statp.tile([128, 1], f32, tag="u")

                def cur_tile(self, it):
                    cur_ps = psAp.tile([128, TS], f32, tag="cur_ps")
                    for k in range(KC):
                        lhs = self.xm[:, it * KC + k, :]
                        nc.tensor.matmul(
                            cur_ps[:], lhs,
                            g_sb[:, (it * KC + k) * TS:(it * KC + k + 1) * TS],
                            start=(k == 0), stop=(k == KC - 1))
                    # drain on ScalarE; sum(A*cur) usually comes from the psO
                    # row sum at store time (telescoping) — except on the
                    # last chunk, where the explicit accum keeps the final
                    # dependency chain off the store path
                    if self.last:
                        nc.scalar.activation(self.combo[:, it, 0:TS], cur_ps[:],
                                             Act.Identity, bias=0.0, scale=1.0,
                                             accum_out=self.csum[:, it:it + 1])
                    else:
                        nc.scalar.activation(self.combo[:, it, 0:TS], cur_ps[:],
                                             Act.Identity, bias=0.0, scale=1.0)

                def idl_tile(self, it):
                    # fp8 DoubleRow: 2 k-tiles per instruction at 0.5
                    # cycles/row; levels 0..15 are exact in e4m3, and the
                    # row-sum (which cannot tolerate fp8 x) is folded into
                    # the host-computed D0 table instead
                    id_ps = psBp.tile([128, TS], f32, tag="id_ps")
                    for j in range(KC // 2):
                        c = it * KC + 2 * j
                        rhs = rl_sb[:, c * TS:(c + 2) * TS].rearrange(
                            "p (t n) -> p t n", t=2)
                        nc.tensor.matmul(
                            id_ps[:], self.xm8[:, c:c + 2, :], rhs,
                            start=(j == 0), stop=(j == KC // 2 - 1),
                            perf_mode=mybir.MatmulPerfMode.DoubleRow)
                    nc.scalar.activation(self.combo[:, it, TS:2 * TS], id_ps[:],
                                         Act.Identity, bias=0.0, scale=1.0)

                def tile(self, it):
                    self.cur_tile(it)
                    self.idl_tile(it)

                def fold_stats(self, lo, w):
                    """Batched max/min over tiles [lo, lo+w): two fp16 2x
                    fold levels on DVE + one finish reduce (contiguous
                    [128, w, 2] output in the interleaved stats buffers),
                    then the per-row coefficient math for that range."""
                    hi = lo + w
                    cvh = self.combo[:, lo:hi, :].rearrange(
                        "p t (h x) -> p t h x", h=2)
                    for op, stat in ((Alu.max, self.smax), (Alu.min, self.smin)):
                        tg = "f" + str(w)
                        f1 = foldp.tile([128, w, 2, H], f16, tag=tg + "1")
                        f2 = foldp.tile([128, w, 2, Q], f16, tag=tg + "2")
                        f3 = foldp.tile([128, w, 2, Q // 2], f16, tag=tg + "3")
                        nc.vector.tensor_tensor(out=f1[:],
                                                in0=cvh[:, :, :, 0:H],
                                                in1=cvh[:, :, :, H:TS],
                                                op=op)
                        nc.vector.tensor_tensor(out=f2[:],
                                                in0=f1[:, :, :, 0:Q],
                                                in1=f1[:, :, :, Q:H],
                                                op=op)
                        nc.vector.tensor_tensor(out=f3[:],
                                                in0=f2[:, :, :, 0:Q // 2],
                                                in1=f2[:, :, :, Q // 2:Q],
                                                op=op)
                        nc.vector.tensor_reduce(stat[:, 2 * lo:2 * hi],
                                                f3[:],
                                                axis=mybir.AxisListType.X,
                                                op=op)
                    s = slice(lo, hi)
                    cmx = self.smax[:, 2 * lo:2 * hi:2]
                    zmx = self.smax[:, 2 * lo + 1:2 * hi:2]
                    cmn = self.smin[:, 2 * lo:2 * hi:2]
                    zmn = self.smin[:, 2 * lo + 1:2 * hi:2]
                    nc.vector.tensor_tensor(out=self.dz[:, s], in0=zmx, in1=zmn,
                                            op=Alu.subtract)
                    # dc = (cmaxP + 2^13*1e-8) - cminP (scaled reference 1e-8)
                    nc.vector.scalar_tensor_tensor(
                        out=self.dc[:, s], in0=cmx,
                        scalar=float(np.float32(CS * 1e-8)), in1=cmn,
                        op0=Alu.add, op1=Alu.subtract)
                    nc.vector.reciprocal(out=self.dc[:, s], in_=self.dc[:, s])
                    nc.vector.tensor_tensor(out=self.co[:, s], in0=self.dz[:, s],
                                            in1=self.dc[:, s], op=Alu.mult)
                    nc.vector.tensor_tensor(out=self.Ab[:, s], in0=self.co[:, s],
                                            in1=kA[:, s], op=Alu.mult)
                    if self.last:
                        # explicit -csum*A/512 term (telescope not used here)
                        nc.vector.tensor_scalar(out=self.A2[:, s],
                                                in0=self.Ab[:, s],
                                                scalar1=float(-1.0 / 512.0),
                                                scalar2=None, op0=Alu.mult)
                        nc.vector.tensor_tensor(out=self.t3[:, s],
                                                in0=self.csum[:, s],
                                                in1=self.A2[:, s], op=Alu.mult)
                        nc.vector.tensor_tensor(out=self.Db[:, s],
                                                in0=self.rs[:, s],
                                                in1=self.t3[:, s], op=Alu.add)

                def finish(self):
                    src_d = self.Db if self.last else self.rs
                    nc.vector.tensor_reduce(self.Dt[:], src_d[:],
                                            axis=mybir.AxisListType.X,
                                            op=Alu.add)

                def diag_mms(self, lo, hi):
                    """Build diag(A) tiles (on Act, which has slack) and run
                    the scaled PSUM accumulation matmuls for tiles [lo, hi)."""
                    if lo == 0:
                        self.diag = diagp.tile([128, IT * 128], f16, tag="diag")
                        self.out_ps = psOp.tile([128, TS], f32, tag="out_ps")
                    for it in range(lo, hi):
                        nc.scalar.activation(
                            self.diag[:, it * 128:(it + 1) * 128], id_sb[:],
                            Act.Identity, bias=0.0,
                            scale=self.Ab[:, it:it + 1])
                    for it in range(lo, hi):
                        nc.tensor.matmul(self.out_ps[:],
                                         self.diag[:, it * 128:(it + 1) * 128],
                                         self.combo[:, it, 0:TS],
                                         start=(it == 0), stop=(it == IT - 1))

                def store(self):
                    if self.last:
                        dvec = self.Dt
                    else:
                        # cmean correction telescopes: row-sum of the
                        # accumulated psO is sum_it A_it * rowsum(cur_it)
                        nc.vector.tensor_reduce(self.u[:], self.out_ps[:],
                                                axis=mybir.AxisListType.X,
                                                op=Alu.add)
                        nc.vector.tensor_scalar(out=self.u[:], in0=self.u[:],
                                                scalar1=float(-1.0 / 512.0),
                                                scalar2=self.Dt[:, 0:1],
                                                op0=Alu.mult, op1=Alu.add)
                        dvec = self.u
                    osb = outp.tile([128, TS], f32, tag="osb")
                    nc.vector.scalar_tensor_tensor(
                        out=osb[:], in0=self.out_ps[:], scalar=dvec[:, 0:1],
                        in1=biasb_sb[:], op0=Alu.add, op1=Alu.add)
                    nc.sync.dma_start(
                        out=out_d.ap()[self.m * 128:(self.m + 1) * 128, :],
                        in_=osb[:])

            def emit_epilogue(cx):
                cx.diag_mms(0, IT)
                cx.store()

            pends = []

            # chunks 0 and 1 are emitted tile-interleaved: while chunk 0
            # stalls on the streaming tables, chunk 1's tiles (whose tables
            # already landed) keep the PE busy
            # chunks 0 and 1 run tile-interleaved in two phases matching the
            # DMA stream: all cur matmuls (g tables), then all ideal matmuls
            # (level tables) — the PE chases the table DMAs without stalling
            c0 = Chunk(0, xm0, xm0_8)
            c1 = Chunk(1, xm1, xm1_8)
            for it in range(IT):
                c0.cur_tile(it)
                c1.cur_tile(it)
            for it in range(IT):
                c0.idl_tile(it)
                c1.idl_tile(it)
                if it == HF - 1:
                    c0.fold_stats(0, HF)
                    c1.fold_stats(0, HF)
            for cx in (c0, c1):
                cx.fold_stats(HF, HF)
                cx.finish()
                pends.append(cx)

            for m in range(2, MB):
                # previous chunks' epilogues emitted FIRST so their ops
                # outrank this chunk's drains/folds once their inputs land
                emit_epilogue(pends.pop(0))
                last = m == MB - 1
                xm = xmp.tile([128, IT * KC, 128], f16, tag="xm")
                nc.sync.dma_start(out=xm[:].rearrange("p c m -> p (c m)"),
                                  in_=xt_d.ap()[m])
                xm8 = xmp.tile([128, IT * KC, 128], f8, tag="xm8")
                nc.sync.dma_start(out=xm8[:].rearrange("p c m -> p (c m)"),
                                  in_=x8_d.ap()[m])
                cx = Chunk(m, xm, xm8, last=last)
                for it in range(IT):
                    cx.tile(it)
                    if it == HF - 1:
                        cx.fold_stats(0, HF)
                        if last:
                            # flush the remaining queued epilogue, then get
                            # the final chunk's first-half accumulation going
                            emit_epilogue(pends.pop(0))
                            cx.diag_mms(0, HF)
                    elif last and it == IT - 3:
                        # ever-finer batches (4,2,1,1) at the end of the
                        # final chunk keep the drain->fold->stats->matmul
                        # tail as short as possible
                        cx.fold_stats(HF, 2)
                        cx.diag_mms(HF, HF + 2)
                    elif last and it == IT - 2:
                        cx.fold_stats(IT - 2, 1)
                        cx.diag_mms(IT - 2, IT - 1)
                cx.fold_stats(IT - 1, 1) if last else cx.fold_stats(HF, HF)
                cx.finish()
                if last:
                    cx.diag_mms(IT - 1, IT)
                    cx.store()
                else:
                    pends.append(cx)
            for cx in pends:
                emit_epilogue(cx)

    nc.compile()
    return nc


def _f32(v):
    return np.float32(v)


def _host_prep(x, weight, bias):
    """Per-core input maps. Weight-static precompute in float32 matching the
    reference op order; conductances shipped as fp16 (scaled by 2^13)."""
    x = np.ascontiguousarray(x, dtype=np.float32)
    weight = np.ascontiguousarray(weight, dtype=np.float32)
    bias = np.ascontiguousarray(bias, dtype=np.float32)

    # x in per-chunk-contiguous layout: xq[m, p, c, col] = x[m*128+col, c*128+p]
    xq16 = np.ascontiguousarray(
        x.reshape(MB, 128, IT * KC, 128).transpose(0, 3, 2, 1)
        .astype(np.float16).reshape(MB, 128, IT * KC * 128))
    import ml_dtypes
    f8t = ml_dtypes.float8_e4m3fn
    xq8 = np.ascontiguousarray(
        x.reshape(MB, 128, IT * KC, 128).transpose(0, 3, 2, 1)
        .astype(f8t).reshape(MB, 128, IT * KC * 128))
    rsum = x.reshape(B, IT, TS).sum(axis=2, dtype=np.float32)   # [1024, 8]

    # weight tiles [it, i, core, j]
    wr = weight.reshape(IT, TS, NCORES, TS)
    wmin = wr.min(axis=(1, 3))                                  # [it, d] f32
    wmax = wr.max(axis=(1, 3))
    gr = _f32(G_MAX) - _f32(G_MIN)
    s = (gr / (wmax - wmin + _f32(1e-12))).astype(np.float32)   # [it, d]
    step = _f32(float(gr) / float(2 ** BITS - 1))

    # cond -> quantized levels -> g_eff, in reference f32 op order
    cond = (wr - wmin[:, None, :, None]) * s[:, None, :, None] + _f32(G_MIN)
    lev = np.round((cond - _f32(G_MIN)) / step).astype(np.float32)
    q = lev * step + _f32(G_MIN)
    i = np.arange(TS, dtype=np.float32)[:, None]                # in-tile row
    j = np.arange(TS, dtype=np.float32)[None, :]
    r_wire = _f32(RP) * ((_f32(TS) - i) + (j + _f32(1.0)))      # [TS, TS]
    g_eff = _f32(1.0) / (_f32(1.0) / q + r_wire[None, :, None, :])
    g16 = (g_eff * _f32(CS)).astype(np.float16)                 # [it, i, d, j]
    rl16 = lev.astype(np.float16)                               # exact 0..15

    def chunkify(a):  # [it, i(=TS), j] -> [128, it*kc*TS]
        return np.ascontiguousarray(
            a.reshape(IT, KC, 128, TS).transpose(2, 0, 1, 3)
            .reshape(128, IT * KC * TS))

    kA = (step / s).astype(np.float32)                          # [it, d]
    kZ = (step / (_f32(512.0) * s)).astype(np.float32)
    ident = np.eye(128, dtype=np.float32)

    # host-exact ideal row sums (fp8 x on device would be far too coarse
    # for the mean term) folded with the offset into one D0 table:
    # D0[row, it, d] = kZ*zsum + wmin*rsum
    levrow = lev.sum(axis=3, dtype=np.float32)                  # [it, i, d]
    zsum = np.einsum("rti,tid->rtd", x.reshape(B, IT, TS), levrow,
                     dtype=np.float32).astype(np.float32)       # [row, it, d]
    d0 = (kZ[None, :, :] * zsum
          + wmin[None, :, :] * rsum[:, :, None]).astype(np.float32)

    in_maps = []
    for d in range(NCORES):
        kon = np.empty((128, 3 * IT), dtype=np.float32)
        kon[:, 0:IT] = kA[:, d][None, :]
        kon[:, IT:2 * IT] = kZ[:, d][None, :]
        kon[:, 2 * IT:3 * IT] = wmin[:, d][None, :]
        d0_r = np.ascontiguousarray(
            d0[:, :, d].reshape(MB, 128, IT).transpose(1, 0, 2)
            .reshape(128, MB * IT), dtype=np.float32)
        in_maps.append({
            "xq16": xq16,
            "xq8": xq8,
            "g16": chunkify(g16[:, :, d, :]),
            "rl8": chunkify(rl16[:, :, d, :]).astype(f8t),
            "rsum": d0_r,
            "kon": kon,
            "biasb": np.ascontiguousarray(
                np.broadcast_to(bias[d * TS:(d + 1) * TS], (128, TS))),
            "ident": ident,
        })
    return in_maps


def get_nc():
    if "nc" not in _CACHE:
        _CACHE["nc"] = _build()
    return _CACHE["nc"]


def kernel(x, weight, bias):
    from concourse.bass_utils import run_bass_kernel_spmd

    nc = get_nc()
    in_maps = _host_prep(x, weight, bias)
    res = run_bass_kernel_spmd(nc, in_maps, core_ids=list(range(NCORES)))
    out = np.empty((B, OUT_F), dtype=np.float32)
    for d in range(NCORES):
        out[:, d * TS:(d + 1) * TS] = res.results[d]["out"]
    return out
